# revision 11
# baseline (speedup 1.0000x reference)
"""Trainium2 Bass kernel for nn_Block_Attention_3 (sparse_attention).

Contract: kernel(**inputs) takes FULL fp32 inputs (as in reference.setup_inputs())
and returns the FULL (4, 2304, 16, 16) fp32 output.

Strategy (zero-collective position sharding + mixed fp8/bf16 precision):
  The image is 16x16 = 4x4 grid of 4x4 patches. All cross-position coupling in
  the block stays within one (batch, patch-row) group, so the 16 units (b, i)
  shard cleanly across 8 cores, 2 units/core, with weights replicated.

  Precision split (validated against the fp32 reference on CPU):
  - The attention-score path is saturated (score sigma ~16, fp8 noise ~1-2
    logits almost never flips the softmax), so x, wd, wk, wq run in
    float8e4m3 with DoubleRow matmuls (0.5 cyc/row) — halving both their HBM
    bytes and their PE time.
  - The V path is the error-sensitive one (V enters the output linearly), so
    the v conv stays bf16 (wv bf16, x bf16).
  Measured end-to-end rel err is identical to the all-bf16 kernel (1.8e-3).

Per-core pipeline (single Bass program, SPMD over 8 cores):
  - inference BN folded into conv weights/biases on host; out-BN scale folded
    into the V path; v-bias and out-BN scale ride the posA operand.
  - pixels laid out patch-major: pix = u*64 + 16*jp + 4*ph + pw.
  - d conv in A-layout [pix, outch] (fp8 DR); k,q convs in B-layout
    [outch, pix] (fp8 DR, weight-pair stationary) so the scores matmul needs
    no transposes; v conv in A-layout (bf16). Biases enter as rank-1 matmuls
    at the END of each PSUM accumulation group.
  - DMA issue is spread across SP + Activation (HWDGE) and Pool (SWDGE) so no
    single sequencer serializes the load stream; stream order follows compute
    order (x8, wd, wk/wq, xb, wv last) and wv's consumer chain is the
    shortest, minimizing the post-stream tail.
  - attention runs as one batched 128x128 matmul pair per half; block-diagonal
    -30000 mask pre-accumulated into the scores PSUM via a single K=9 matmul;
    outputs written back in bf16 (host upcasts).
"""
import os
import sys

sys.path.insert(0, "/opt/trn_rl_repo")

import numpy as np

EPS = 1e-5
D_IN, D, B, HW, P = 2048, 256, 4, 16, 4
NCHUNK = D_IN // 128   # 16
NPAIR = NCHUNK // 2    # 8 chunk-pairs for DoubleRow
N_CORES = 8
MASK_NEG = 30000.0

_CACHE = {}

# rows aux layout (bf16): [1, 1152]
_R_ONES = slice(0, 128)
_R_BD = slice(128, 384)            # d-conv bias (BN-folded)
_R_BETA = slice(384, 640)          # out-BN beta
_R_BKQ = slice(640, 1152)          # bk0|bk1|bq0|bq1 rows [1,128] each
ROWS_LEN = 1152

# combo layout (bf16): posA(+bv, so-scaled)[0:256] | posb[256:512] | R[512:520]
COMBO_LEN = 520


def _build_program(tag="mixed_fp8"):
    """Build (and compile to BIR) the single-core SPMD Bass program."""
    import concourse.mybir as mybir
    import concourse.tile as tile
    from concourse import bacc

    bf = mybir.dt.bfloat16
    f8 = mybir.dt.float8e4
    f32 = mybir.dt.float32
    DR = mybir.MatmulPerfMode.DoubleRow

    nc = bacc.Bacc("TRN2", target_bir_lowering=False, debug=False,
                   num_devices=N_CORES)

    x8_d = nc.dram_tensor("x8", [128, NCHUNK * 128], f8, kind="ExternalInput")
    xb_d = nc.dram_tensor("xb", [128, NCHUNK * 128], bf, kind="ExternalInput")
    wd_d = nc.dram_tensor("wd", [128, NCHUNK * 256], f8, kind="ExternalInput")
    wk_d = nc.dram_tensor("wk", [128, NCHUNK * 256], f8, kind="ExternalInput")
    wq_d = nc.dram_tensor("wq", [128, NCHUNK * 256], f8, kind="ExternalInput")
    wv_d = nc.dram_tensor("wv", [128, NCHUNK * 256], bf, kind="ExternalInput")
    combo_d = nc.dram_tensor("combo", [128, COMBO_LEN], bf, kind="ExternalInput")
    rows_d = nc.dram_tensor("rows", [1, ROWS_LEN], bf, kind="ExternalInput")
    mask9_d = nc.dram_tensor("mask9", [9, 256], bf, kind="ExternalInput")
    out_d = nc.dram_tensor("xloc", [128, 256], bf, kind="ExternalOutput")

    with tile.TileContext(nc) as tc:
        with (
            tc.tile_pool(name="big", bufs=1) as big,
            tc.tile_pool(name="small", bufs=1) as small,
            tc.tile_pool(name="ps", bufs=1, space="PSUM") as ps,
            tc.tile_pool(name="ps2", bufs=2, space="PSUM") as ps2,
        ):
            x8t = big.tile([128, NCHUNK * 128], f8, tag="x8t")
            xbt = big.tile([128, NCHUNK * 128], bf, tag="xbt")
            wdt = big.tile([128, NCHUNK * 256], f8, tag="wdt")
            wkt = big.tile([128, NCHUNK * 256], f8, tag="wkt")
            wqt = big.tile([128, NCHUNK * 256], f8, tag="wqt")
            wvt = big.tile([128, NCHUNK * 256], bf, tag="wvt")
            combo = small.tile([128, COMBO_LEN], bf, tag="combo")
            rows = small.tile([1, ROWS_LEN], bf, tag="rows")
            mask9 = small.tile([9, 256], bf, tag="mask9")

            # ---- DMA loads, engine-spread. Global arrival order (alternating
            # SP / Act so HWDGE serves them in compute order):
            #   x8h0, x8h1, wd0, wq0, wd1, wq1, wk0, wk1, xb0, xb1,
            #   wv[och0], wv[och1 c0-7], wv[och1 c8-15]
            # Pool (SWDGE): small aux tensors (combo/rows/mask9).
            h8 = (NCHUNK * 128) // 2   # 1024 cols (half of x)
            hw_ = (NCHUNK * 256) // 2  # 2048 cols (half of a weight)
            qw = hw_ // 2              # 1024 cols (quarter of a weight)
            nc.sync.dma_start(x8t[:], x8_d.ap())
            nc.scalar.dma_start(wdt[:, 0:hw_], wd_d.ap()[:, 0:hw_])
            nc.sync.dma_start(wkt[:, 0:hw_], wk_d.ap()[:, 0:hw_])
            nc.scalar.dma_start(wdt[:, hw_:2 * hw_], wd_d.ap()[:, hw_:2 * hw_])
            nc.sync.dma_start(wkt[:, hw_:2 * hw_], wk_d.ap()[:, hw_:2 * hw_])
            nc.scalar.dma_start(wqt[:, 0:hw_], wq_d.ap()[:, 0:hw_])
            nc.sync.dma_start(xbt[:, 0:h8], xb_d.ap()[:, 0:h8])
            nc.scalar.dma_start(wqt[:, hw_:2 * hw_], wq_d.ap()[:, hw_:2 * hw_])
            nc.sync.dma_start(xbt[:, h8:2 * h8], xb_d.ap()[:, h8:2 * h8])
            ew = qw // 2               # 512 cols (eighth of a weight)
            nc.sync.dma_start(wvt[:, 0:hw_], wv_d.ap()[:, 0:hw_])
            nc.sync.dma_start(wvt[:, hw_:hw_ + qw], wv_d.ap()[:, hw_:hw_ + qw])
            nc.sync.dma_start(wvt[:, hw_ + qw:hw_ + qw + ew],
                              wv_d.ap()[:, hw_ + qw:hw_ + qw + ew])
            nc.sync.dma_start(wvt[:, hw_ + qw + ew:2 * hw_],
                              wv_d.ap()[:, hw_ + qw + ew:2 * hw_])
            nc.gpsimd.dma_start(combo[:], combo_d.ap())
            nc.gpsimd.dma_start(rows[:], rows_d.ap())
            nc.gpsimd.dma_start(mask9[:], mask9_d.ap())

            posa = combo[:, 0:256]
            posb = combo[:, 256:512]
            R_ap = combo[:, 512:520]
            ones_r = rows[0:1, _R_ONES]

            # ---- conv PSUM accumulators ----
            d_ps = ps2.tile([128, 256], f32, tag="post", name="d_ps")
            kq_ps = [[ps.tile([128, 128], f32, tag=f"{n}{h}_ps", name=f"{n}{h}_ps")
                      for h in range(2)] for n in ("k", "q")]
            v_ps = [ps.tile([128, 128], f32, tag=f"v{g}_ps", name=f"v{g}_ps")
                    for g in range(2)]

            def x8pair(cp):
                return x8t[:, cp * 256:(cp + 1) * 256].rearrange(
                    "p (t j) -> p t j", t=2)

            # d conv: A-layout [pix, outch], fp8 DoubleRow, x pair stationary
            for cp in range(NPAIR):
                nc.tensor.matmul(
                    d_ps[:], x8pair(cp),
                    wdt[:, cp * 512:(cp + 1) * 512].rearrange(
                        "p (t o) -> p t o", t=2),
                    start=(cp == 0), stop=False, perf_mode=DR)
            nc.tensor.matmul(d_ps[:], ones_r, rows[0:1, _R_BD],
                             start=False, stop=True)

            # k,q convs: B-layout [outch, pix], fp8 DoubleRow, w pair stationary
            def kq_conv(wt, wi, h):
                acc = kq_ps[wi][h]
                boff = _R_BKQ.start + (wi * 2 + h) * 128
                for cp in range(NPAIR):
                    nc.tensor.matmul(
                        acc[:],
                        wt[:, h * 2048 + cp * 256:h * 2048 + (cp + 1) * 256]
                        .rearrange("p (t o) -> p t o", t=2),
                        x8pair(cp),
                        start=(cp == 0), stop=False, perf_mode=DR)
                nc.tensor.matmul(acc[:], rows[0:1, boff:boff + 128], ones_r,
                                 start=False, stop=True)

            kq_conv(wkt, 0, 0)
            kq_conv(wkt, 0, 1)
            kq_conv(wqt, 1, 0)
            kq_conv(wqt, 1, 1)

            # ---- d path: relu -> exp(accum) -> normalize -> S = incx.T @ R ----
            AF = mybir.ActivationFunctionType
            inc = small.tile([128, 256], f32, tag="inc")
            nc.scalar.activation(inc[:], d_ps[:], AF.Relu)
            einc = small.tile([128, 256], f32, tag="einc")
            dsum = small.tile([128, 1], f32, tag="dsum")
            nc.scalar.activation(einc[:], inc[:], AF.Exp, accum_out=dsum[:])
            dsuminv = small.tile([128, 1], f32, tag="dsuminv")
            nc.vector.reciprocal(dsuminv[:], dsum[:])
            incx = small.tile([128, 256], bf, tag="incx")
            nc.vector.tensor_scalar_mul(incx[:], einc[:], dsuminv[:, 0:1])
            s_ps = ps2.tile([128, 16], f32, tag="post", name="s_ps")
            sT = small.tile([128, 16], f32, tag="sT")
            for h in range(2):
                nc.tensor.matmul(s_ps[:, h * 8:(h + 1) * 8],
                                 incx[:, h * 128:(h + 1) * 128],
                                 R_ap, start=True, stop=True,
                                 skip_group_check=(h == 1))
            nc.vector.tensor_copy(sT[:], s_ps[:])

            # ---- scores PSUM: block mask first (one K=9 matmul), then Kp.T@J
            sc_ps = ps2.tile([128, 128], f32, tag="post", name="sc_ps")
            nc.tensor.matmul(sc_ps[:], mask9[:, 0:128], mask9[:, 128:256],
                             start=True, stop=False)

            # ---- k,q paths (B-layout): Kp = psum + pos (Pool); J = psum*S +
            # pos (DVE); then the scores matmuls ----
            kp = [small.tile([128, 128], bf, tag=f"kp{h}", name=f"kp{h}")
                  for h in range(2)]
            jp = [small.tile([128, 128], bf, tag=f"jp{h}", name=f"jp{h}")
                  for h in range(2)]
            jtmp = [small.tile([128, 128], f32, tag=f"jtmp{h}", name=f"jtmp{h}")
                    for h in range(2)]
            for h in range(2):
                nc.gpsimd.tensor_tensor(kp[h][:], kq_ps[0][h][:],
                                        posb[:, h * 128:(h + 1) * 128],
                                        op=mybir.AluOpType.add)
                s_bcast = sT[:, h * 8:(h + 1) * 8].unsqueeze(2).broadcast_to((128, 8, 16))
                q3d = kq_ps[1][h][:].rearrange("p (b w) -> p b w", b=8)
                j3d = jtmp[h][:].rearrange("p (b w) -> p b w", b=8)
                nc.vector.tensor_tensor(j3d, q3d, s_bcast, op=mybir.AluOpType.mult)
                nc.vector.tensor_tensor(jp[h][:], jtmp[h][:],
                                        posb[:, h * 128:(h + 1) * 128],
                                        op=mybir.AluOpType.add)
                # High priority: the tile scheduler must NOT park these behind
                # the v-conv matmuls (which wait on the late wv stream) in the
                # in-order PE stream — the whole softmax chain hangs off them.
                with tc.high_priority():
                    nc.tensor.matmul(sc_ps[:], kp[h][:], jp[h][:],
                                     start=False, stop=(h == 1))

            # ---- att softmax over free dim (queries n) ----
            nmx = small.tile([128, 1], f32, tag="nmx")
            nc.vector.reduce_max(nmx[:], sc_ps[:], axis=mybir.AxisListType.X,
                                 negate=True)
            e_t = small.tile([128, 128], f32, tag="e_t")
            den = small.tile([128, 1], f32, tag="den")
            nc.scalar.activation(e_t[:], sc_ps[:], AF.Exp, bias=nmx[:, 0:1],
                                 accum_out=den[:])
            deninv = small.tile([128, 1], f32, tag="deninv")
            nc.vector.reciprocal(deninv[:], den[:])
            att = small.tile([128, 128], f32, tag="att")
            nc.vector.tensor_scalar_mul(att[:], e_t[:], deninv[:, 0:1])

            # ---- v halves (streamed last, bf16): per-half conv -> vpt ->
            # V-matmul -> bf16 copy -> DMA out; half 0 completes while wv
            # half 1 still streams. att-bias matmuls run early (dep-free). ----
            vpt = small.tile([128, 256], f32, tag="vpt")
            xloc = small.tile([128, 256], bf, tag="xloc")
            att_ps = [ps2.tile([128, 128], f32, tag="post", name=f"att_ps{g}")
                      for g in range(2)]
            for g in range(2):
                nc.tensor.matmul(att_ps[g][:], ones_r,
                                 rows[0:1, 384 + g * 128:384 + (g + 1) * 128],
                                 start=True, stop=False)
            for g in range(2):
                gs = slice(g * 128, (g + 1) * 128)
                # Logical wait pushes the v-conv matmuls AFTER the scores
                # matmuls in the scheduler's PE stream: the scheduler
                # mis-estimates wv's DMA arrival as early and would otherwise
                # park the (earlier-ready) scores chain behind them.
                with tc.tile_wait_until(0.008 + 0.002 * g):
                    for c in range(NCHUNK):
                        nc.tensor.matmul(
                            v_ps[g][:], xbt[:, c * 128:(c + 1) * 128],
                            wvt[:, g * 2048 + c * 128:g * 2048 + (c + 1) * 128],
                            start=(c == 0), stop=(c == NCHUNK - 1))
                veng = nc.gpsimd if g == 0 else nc.vector
                veng.tensor_tensor(vpt[:, gs], v_ps[g][:], posa[:, gs],
                                   op=mybir.AluOpType.add)
                nc.tensor.matmul(att_ps[g][:], att[:], vpt[:, gs],
                                 start=False, stop=True)
                nc.vector.tensor_copy(xloc[:, gs], att_ps[g][:])
                # out0 via Pool SWDGE: keeps the shared HWDGE free so the
                # final out1 DMA is not queued behind out0's HWDGE slot.
                eng = nc.gpsimd if g == 0 else nc.sync
                eng.dma_start(out_d.ap()[:, gs], xloc[:, gs])

    nc.compile()
    return nc


def _fold_bn(w, b, g, beta, m, v):
    s = g / np.sqrt(v + EPS)
    return (w * s[:, None]).astype(np.float32), (s * (b - m) + beta).astype(np.float32)


def _prep(inputs):
    """Host-side prep: BN folds, fp8/bf16 packing, per-core input maps."""
    import ml_dtypes
    bf = ml_dtypes.bfloat16
    f8 = ml_dtypes.float8_e4m3

    inp = {k: np.asarray(v, dtype=np.float32) for k, v in inputs.items()}
    x, pos = inp["x"], inp["pos"]
    wk, bk = _fold_bn(inp["wk"], inp["bk"], inp["gk"], inp["betak"], inp["mk"], inp["vk"])
    wq, bq = _fold_bn(inp["wq"], inp["bq"], inp["gq"], inp["betaq"], inp["mq"], inp["vq"])
    wv, bv = _fold_bn(inp["wv"], inp["bv"], inp["gv"], inp["betav"], inp["mv"], inp["vv"])
    wd, bd = _fold_bn(inp["wd"], inp["bd"], inp["gd"], inp["betad"], inp["md"], inp["vd"])
    so = (inp["go"] / np.sqrt(inp["vo"] + EPS)).astype(np.float32)
    beta_o = (inp["beto"] - inp["mo"] * so).astype(np.float32)
    wv = wv * so[:, None]
    bv = bv * so  # folded into posA below

    def pack_dr_a(w):
        # A-layout rhs for DoubleRow: [p, (cp, t, 256o)]; w is [256o, 2048in]
        wt = w.T.reshape(NPAIR, 2, 128, 256).transpose(2, 0, 1, 3).reshape(128, -1)
        return np.ascontiguousarray(wt).astype(f8)

    def pack_dr_b(w):
        # B-layout lhsT for DoubleRow: [p, (h, cp, t, 128o)]; w is [256o, 2048in]
        wt = w.T.reshape(NPAIR, 2, 128, 2, 128).transpose(2, 3, 0, 1, 4).reshape(128, -1)
        return np.ascontiguousarray(wt).astype(f8)

    def pack_hmaj(w):
        # outch-half-major bf16 (v conv): [p, (h, c, 128o)]
        wt = w.T.reshape(NCHUNK, 128, 2, 128).transpose(1, 2, 0, 3).reshape(128, -1)
        return np.ascontiguousarray(wt).astype(bf)

    w_packed = {"wd": pack_dr_a(wd), "wk": pack_dr_b(wk), "wq": pack_dr_b(wq),
                "wv": pack_hmaj(wv)}

    p_idx = np.arange(128)
    R = np.zeros((128, 8), np.float32)
    R[p_idx, (p_idx // 64) * 4 + (p_idx % 16) // 4] = 1.0
    pix_patch = (p_idx // 64) * 4 + (p_idx % 64) // 16
    blk_ind = (pix_patch[None, :] == np.arange(8)[:, None]).astype(np.float32)

    rows = np.zeros((1, ROWS_LEN), np.float32)
    rows[0, _R_ONES] = 1.0
    rows[0, _R_BD] = bd
    rows[0, _R_BETA] = beta_o
    rows[0, _R_BKQ] = np.concatenate([bk, bq])
    rows = rows.astype(bf)

    mask9 = np.zeros((9, 256), np.float32)
    mask9[0, 0:128] = 1.0
    mask9[0, 128:256] = -MASK_NEG
    mask9[1:9, 0:128] = blk_ind
    mask9[1:9, 128:256] = blk_ind * MASK_NEG
    mask9 = mask9.astype(bf)

    units = [(b, i) for b in range(B) for i in range(P)]
    in_maps = []
    for core in range(N_CORES):
        cu = units[2 * core:2 * core + 2]
        x_sb = np.empty((128, NCHUNK, 128), np.float32)
        pos_A = np.empty((128, 256), np.float32)
        posb_sb = np.empty((128, 256), np.float32)
        for u, (b, i) in enumerate(cu):
            # [c, ph, jp, pw] -> patch-major pixel (jp, ph, pw)
            xs = x[b, :, 4 * i:4 * i + 4, :].reshape(D_IN, 4, 4, 4)
            xs = xs.transpose(0, 2, 1, 3).reshape(D_IN, 64)
            x_sb[:, :, 64 * u:64 * u + 64] = xs.reshape(NCHUNK, 128, 64).transpose(1, 0, 2)
            ps_ = pos[b, :, 4 * i:4 * i + 4, :].reshape(D, 4, 4, 4).transpose(0, 2, 1, 3).reshape(D, 64)
            pos_A[64 * u:64 * u + 64, :] = ps_.T
            posb_sb[:, 64 * u:64 * u + 64] = ps_[0:128]
            posb_sb[:, 128 + 64 * u:128 + 64 * u + 64] = ps_[128:256]
        pos_A_sov = (pos_A * so[None, :] + bv[None, :]).astype(np.float32)
        xb = np.ascontiguousarray(x_sb.reshape(128, -1)).astype(bf)
        x8 = xb.astype(f8)  # fp8(bf16(x)): matches the CPU-validated chain
        combo = np.concatenate(
            [pos_A_sov, posb_sb, R], axis=1).astype(bf)
        in_maps.append({
            "x8": x8, "xb": xb,
            "wd": w_packed["wd"], "wk": w_packed["wk"],
            "wq": w_packed["wq"], "wv": w_packed["wv"],
            "combo": combo, "rows": rows, "mask9": mask9,
        })
    return in_maps, units


def _run_device(nc, in_maps):
    from concourse.bass_utils import run_bass_kernel_spmd
    return run_bass_kernel_spmd(nc, in_maps, list(range(N_CORES))).results


def _subproc_main(inp_path, out_path):
    import pickle
    with open(inp_path, "rb") as f:
        in_maps = pickle.load(f)
    nc = _build_program()
    res = _run_device(nc, in_maps)
    with open(out_path, "wb") as f:
        pickle.dump(res, f)


def _run_via_subprocess(in_maps):
    import pickle
    import subprocess
    import tempfile
    here = os.path.dirname(os.path.abspath(__file__))
    last = None
    for _ in range(2):
        with tempfile.TemporaryDirectory() as td:
            inp = os.path.join(td, "in.pkl")
            outp = os.path.join(td, "out.pkl")
            with open(inp, "wb") as f:
                pickle.dump(in_maps, f)
            code = (f"import sys; sys.path.insert(0, {here!r}); "
                    f"import kernel; kernel._subproc_main({inp!r}, {outp!r})")
            try:
                r = subprocess.run([sys.executable, "-c", code], timeout=1800)
                if r.returncode == 0 and os.path.exists(outp):
                    with open(outp, "rb") as f:
                        return pickle.load(f)
                last = RuntimeError(f"subprocess rc={r.returncode}")
            except Exception as e:  # noqa: BLE001
                last = e
    raise RuntimeError(f"device execution failed after retries: {last}")


def kernel(**inputs) -> np.ndarray:
    key = ("prog", "mixed_fp8")
    if key not in _CACHE:
        _CACHE[key] = _build_program()
    nc = _CACHE[key]

    in_maps, units = _prep(inputs)
    try:
        results = _run_device(nc, in_maps)
    except Exception:
        # A crashed NEFF execution can poison this process's jax runtime
        # (NRT_EXEC_UNIT_UNRECOVERABLE); a fresh process recovers reliably.
        results = _run_via_subprocess(in_maps)

    x_loc = np.zeros((B, D, HW, HW), np.float32)
    for core in range(N_CORES):
        xl = np.asarray(results[core]["xloc"], dtype=np.float32)  # [128 pix, 256 c]
        for u, (b, i) in enumerate(units[2 * core:2 * core + 2]):
            blk = xl[64 * u:64 * u + 64, :].reshape(4, 4, 4, D).transpose(3, 1, 0, 2)
            x_loc[b, :, 4 * i:4 * i + 4, :] = blk.reshape(D, 4, 16)
    return np.concatenate([np.asarray(inputs["x"], np.float32), x_loc], axis=1)


# revision 12
# speedup vs baseline: 1.0158x; 1.0158x over previous
"""Trainium2 Bass kernel for nn_Block_Attention_3 (sparse_attention).

Contract: kernel(**inputs) takes FULL fp32 inputs (as in reference.setup_inputs())
and returns the FULL (4, 2304, 16, 16) fp32 output.

Strategy (zero-collective position sharding + mixed fp8/bf16 precision):
  The image is 16x16 = 4x4 grid of 4x4 patches. All cross-position coupling in
  the block stays within one (batch, patch-row) group, so the 16 units (b, i)
  shard cleanly across 8 cores, 2 units/core, with weights replicated.

  Precision split (validated against the fp32 reference on CPU):
  - The attention-score path is saturated (score sigma ~16, fp8 noise ~1-2
    logits almost never flips the softmax), so x, wd, wk, wq run in
    float8e4m3 with DoubleRow matmuls (0.5 cyc/row) — halving both their HBM
    bytes and their PE time.
  - The V path is the error-sensitive one (V enters the output linearly), so
    the v conv stays bf16 (wv bf16, x bf16).
  Measured end-to-end rel err is identical to the all-bf16 kernel (1.8e-3).

Per-core pipeline (single Bass program, SPMD over 8 cores):
  - inference BN folded into conv weights/biases on host; out-BN scale folded
    into the V path; v-bias and out-BN scale ride the posA operand.
  - pixels laid out patch-major: pix = u*64 + 16*jp + 4*ph + pw.
  - d conv in A-layout [pix, outch] (fp8 DR); k,q convs in B-layout
    [outch, pix] (fp8 DR, weight-pair stationary) so the scores matmul needs
    no transposes; v conv in A-layout (bf16). Biases enter as rank-1 matmuls
    at the END of each PSUM accumulation group.
  - DMA issue is spread across SP + Activation (HWDGE) and Pool (SWDGE) so no
    single sequencer serializes the load stream; stream order follows compute
    order (x8, wd, wk/wq, xb, wv last) and wv's consumer chain is the
    shortest, minimizing the post-stream tail.
  - attention runs as one batched 128x128 matmul pair per half; block-diagonal
    -30000 mask pre-accumulated into the scores PSUM via a single K=9 matmul;
    outputs written back in bf16 (host upcasts).
"""
import os
import sys

sys.path.insert(0, "/opt/trn_rl_repo")

import numpy as np

EPS = 1e-5
D_IN, D, B, HW, P = 2048, 256, 4, 16, 4
NCHUNK = D_IN // 128   # 16
NPAIR = NCHUNK // 2    # 8 chunk-pairs for DoubleRow
N_CORES = 8
MASK_NEG = 30000.0

_CACHE = {}

# rows aux layout (bf16): [1, 1152]
_R_ONES = slice(0, 128)
_R_BD = slice(128, 384)            # d-conv bias (BN-folded)
_R_BETA = slice(384, 640)          # out-BN beta
_R_BKQ = slice(640, 1152)          # bk0|bk1|bq0|bq1 rows [1,128] each
ROWS_LEN = 1152

# combo layout (bf16): posA(+bv, so-scaled)[0:256] | posb[256:512] | R[512:520]
COMBO_LEN = 520


def _build_program(tag="mixed_fp8"):
    """Build (and compile to BIR) the single-core SPMD Bass program."""
    import concourse.mybir as mybir
    import concourse.tile as tile
    from concourse import bacc

    bf = mybir.dt.bfloat16
    f8 = mybir.dt.float8e4
    f32 = mybir.dt.float32
    DR = mybir.MatmulPerfMode.DoubleRow

    nc = bacc.Bacc("TRN2", target_bir_lowering=False, debug=False,
                   num_devices=N_CORES)

    x8_d = nc.dram_tensor("x8", [128, NCHUNK * 128], f8, kind="ExternalInput")
    xb_d = nc.dram_tensor("xb", [128, NCHUNK * 128], bf, kind="ExternalInput")
    wd_d = nc.dram_tensor("wd", [128, NCHUNK * 256], f8, kind="ExternalInput")
    wk_d = nc.dram_tensor("wk", [128, NCHUNK * 256], f8, kind="ExternalInput")
    wq_d = nc.dram_tensor("wq", [128, NCHUNK * 256], f8, kind="ExternalInput")
    wv_d = nc.dram_tensor("wv", [128, NCHUNK * 256], bf, kind="ExternalInput")
    combo_d = nc.dram_tensor("combo", [128, COMBO_LEN], bf, kind="ExternalInput")
    rows_d = nc.dram_tensor("rows", [1, ROWS_LEN], bf, kind="ExternalInput")
    mask9_d = nc.dram_tensor("mask9", [9, 256], bf, kind="ExternalInput")
    out_d = nc.dram_tensor("xloc", [128, 256], bf, kind="ExternalOutput")

    with tile.TileContext(nc) as tc:
        with (
            tc.tile_pool(name="big", bufs=1) as big,
            tc.tile_pool(name="small", bufs=1) as small,
            tc.tile_pool(name="ps", bufs=1, space="PSUM") as ps,
            tc.tile_pool(name="ps2", bufs=2, space="PSUM") as ps2,
        ):
            x8t = big.tile([128, NCHUNK * 128], f8, tag="x8t")
            xbt = big.tile([128, NCHUNK * 128], bf, tag="xbt")
            wdt = big.tile([128, NCHUNK * 256], f8, tag="wdt")
            wkt = big.tile([128, NCHUNK * 256], f8, tag="wkt")
            wqt = big.tile([128, NCHUNK * 256], f8, tag="wqt")
            wvt = big.tile([128, NCHUNK * 256], bf, tag="wvt")
            combo = small.tile([128, COMBO_LEN], bf, tag="combo")
            rows = small.tile([1, ROWS_LEN], bf, tag="rows")
            mask9 = small.tile([9, 256], bf, tag="mask9")

            # ---- DMA loads, engine-spread. Global arrival order (alternating
            # SP / Act so HWDGE serves them in compute order):
            #   x8h0, x8h1, wd0, wq0, wd1, wq1, wk0, wk1, xb0, xb1,
            #   wv[och0], wv[och1 c0-7], wv[och1 c8-15]
            # Pool (SWDGE): small aux tensors (combo/rows/mask9).
            h8 = (NCHUNK * 128) // 2   # 1024 cols (half of x)
            hw_ = (NCHUNK * 256) // 2  # 2048 cols (half of a weight)
            qw = hw_ // 2              # 1024 cols (quarter of a weight)
            nc.sync.dma_start(x8t[:], x8_d.ap())
            nc.scalar.dma_start(wdt[:, 0:hw_], wd_d.ap()[:, 0:hw_])
            nc.sync.dma_start(wkt[:, 0:hw_], wk_d.ap()[:, 0:hw_])
            nc.scalar.dma_start(wdt[:, hw_:2 * hw_], wd_d.ap()[:, hw_:2 * hw_])
            nc.sync.dma_start(wkt[:, hw_:2 * hw_], wk_d.ap()[:, hw_:2 * hw_])
            nc.scalar.dma_start(wqt[:, 0:hw_], wq_d.ap()[:, 0:hw_])
            nc.sync.dma_start(xbt[:, 0:h8], xb_d.ap()[:, 0:h8])
            nc.scalar.dma_start(wqt[:, hw_:2 * hw_], wq_d.ap()[:, hw_:2 * hw_])
            nc.sync.dma_start(xbt[:, h8:2 * h8], xb_d.ap()[:, h8:2 * h8])
            ew = qw // 2               # 512 cols (eighth of a weight)
            nc.sync.dma_start(wvt[:, 0:hw_], wv_d.ap()[:, 0:hw_])
            nc.sync.dma_start(wvt[:, hw_:hw_ + qw], wv_d.ap()[:, hw_:hw_ + qw])
            nc.sync.dma_start(wvt[:, hw_ + qw:hw_ + qw + ew],
                              wv_d.ap()[:, hw_ + qw:hw_ + qw + ew])
            nc.sync.dma_start(wvt[:, hw_ + qw + ew:2 * hw_],
                              wv_d.ap()[:, hw_ + qw + ew:2 * hw_])
            nc.gpsimd.dma_start(combo[:], combo_d.ap())
            nc.gpsimd.dma_start(rows[:], rows_d.ap())
            nc.gpsimd.dma_start(mask9[:], mask9_d.ap())

            posa = combo[:, 0:256]
            posb = combo[:, 256:512]
            R_ap = combo[:, 512:520]
            ones_r = rows[0:1, _R_ONES]

            # ---- conv PSUM accumulators ----
            d_ps = ps2.tile([128, 256], f32, tag="post", name="d_ps")
            kq_ps = [[ps.tile([128, 128], f32, tag=f"{n}{h}_ps", name=f"{n}{h}_ps")
                      for h in range(2)] for n in ("k", "q")]
            v_ps = [ps.tile([128, 128], f32, tag=f"v{g}_ps", name=f"v{g}_ps")
                    for g in range(2)]

            def x8pair(cp):
                return x8t[:, cp * 256:(cp + 1) * 256].rearrange(
                    "p (t j) -> p t j", t=2)

            # d conv: A-layout [pix, outch], fp8 DoubleRow, x pair stationary
            for cp in range(NPAIR):
                nc.tensor.matmul(
                    d_ps[:], x8pair(cp),
                    wdt[:, cp * 512:(cp + 1) * 512].rearrange(
                        "p (t o) -> p t o", t=2),
                    start=(cp == 0), stop=False, perf_mode=DR)
            nc.tensor.matmul(d_ps[:], ones_r, rows[0:1, _R_BD],
                             start=False, stop=True)

            # k,q convs: B-layout [outch, pix], fp8 DoubleRow, w pair stationary
            def kq_conv(wt, wi, h):
                acc = kq_ps[wi][h]
                boff = _R_BKQ.start + (wi * 2 + h) * 128
                for cp in range(NPAIR):
                    nc.tensor.matmul(
                        acc[:],
                        wt[:, h * 2048 + cp * 256:h * 2048 + (cp + 1) * 256]
                        .rearrange("p (t o) -> p t o", t=2),
                        x8pair(cp),
                        start=(cp == 0), stop=False, perf_mode=DR)
                nc.tensor.matmul(acc[:], rows[0:1, boff:boff + 128], ones_r,
                                 start=False, stop=True)

            kq_conv(wkt, 0, 0)
            kq_conv(wkt, 0, 1)
            kq_conv(wqt, 1, 0)
            kq_conv(wqt, 1, 1)

            # ---- d path: relu -> exp(accum) -> normalize -> S = incx.T @ R ----
            AF = mybir.ActivationFunctionType
            inc = small.tile([128, 256], f32, tag="inc")
            nc.scalar.activation(inc[:], d_ps[:], AF.Relu)
            einc = small.tile([128, 256], f32, tag="einc")
            dsum = small.tile([128, 1], f32, tag="dsum")
            nc.scalar.activation(einc[:], inc[:], AF.Exp, accum_out=dsum[:])
            dsuminv = small.tile([128, 1], f32, tag="dsuminv")
            nc.vector.reciprocal(dsuminv[:], dsum[:])
            incx = small.tile([128, 256], bf, tag="incx")
            nc.vector.tensor_scalar_mul(incx[:], einc[:], dsuminv[:, 0:1])
            s_ps = ps2.tile([128, 16], f32, tag="post", name="s_ps")
            sT = small.tile([128, 16], f32, tag="sT")
            for h in range(2):
                nc.tensor.matmul(s_ps[:, h * 8:(h + 1) * 8],
                                 incx[:, h * 128:(h + 1) * 128],
                                 R_ap, start=True, stop=True,
                                 skip_group_check=(h == 1))
            nc.vector.tensor_copy(sT[:], s_ps[:])

            # ---- scores PSUM: block mask first (one K=9 matmul), then Kp.T@J
            sc_ps = ps2.tile([128, 128], f32, tag="post", name="sc_ps")
            nc.tensor.matmul(sc_ps[:], mask9[:, 0:128], mask9[:, 128:256],
                             start=True, stop=False)

            # ---- k,q paths (B-layout): Kp = psum + pos (Pool); J = psum*S +
            # pos (DVE); then the scores matmuls ----
            kp = [small.tile([128, 128], bf, tag=f"kp{h}", name=f"kp{h}")
                  for h in range(2)]
            jp = [small.tile([128, 128], bf, tag=f"jp{h}", name=f"jp{h}")
                  for h in range(2)]
            jtmp = [small.tile([128, 128], f32, tag=f"jtmp{h}", name=f"jtmp{h}")
                    for h in range(2)]
            for h in range(2):
                nc.gpsimd.tensor_tensor(kp[h][:], kq_ps[0][h][:],
                                        posb[:, h * 128:(h + 1) * 128],
                                        op=mybir.AluOpType.add)
                s_bcast = sT[:, h * 8:(h + 1) * 8].unsqueeze(2).broadcast_to((128, 8, 16))
                q3d = kq_ps[1][h][:].rearrange("p (b w) -> p b w", b=8)
                j3d = jtmp[h][:].rearrange("p (b w) -> p b w", b=8)
                nc.vector.tensor_tensor(j3d, q3d, s_bcast, op=mybir.AluOpType.mult)
                nc.vector.tensor_tensor(jp[h][:], jtmp[h][:],
                                        posb[:, h * 128:(h + 1) * 128],
                                        op=mybir.AluOpType.add)
                # High priority: the tile scheduler must NOT park these behind
                # the v-conv matmuls (which wait on the late wv stream) in the
                # in-order PE stream — the whole softmax chain hangs off them.
                with tc.high_priority():
                    nc.tensor.matmul(sc_ps[:], kp[h][:], jp[h][:],
                                     start=False, stop=(h == 1))

            # ---- att softmax over free dim (queries n) ----
            nmx = small.tile([128, 1], f32, tag="nmx")
            nc.vector.reduce_max(nmx[:], sc_ps[:], axis=mybir.AxisListType.X,
                                 negate=True)
            e_t = small.tile([128, 128], f32, tag="e_t")
            den = small.tile([128, 1], f32, tag="den")
            nc.scalar.activation(e_t[:], sc_ps[:], AF.Exp, bias=nmx[:, 0:1],
                                 accum_out=den[:])
            deninv = small.tile([128, 1], f32, tag="deninv")
            nc.vector.reciprocal(deninv[:], den[:])
            att = small.tile([128, 128], f32, tag="att")
            nc.vector.tensor_scalar_mul(att[:], e_t[:], deninv[:, 0:1])

            # ---- v halves (streamed last, bf16): per-half conv -> vpt ->
            # V-matmul -> bf16 copy -> DMA out; half 0 completes while wv
            # half 1 still streams. att-bias matmuls run early (dep-free). ----
            vpt = small.tile([128, 256], f32, tag="vpt")
            xloc = small.tile([128, 256], bf, tag="xloc")
            att_ps = [ps2.tile([128, 128], f32, tag="post", name=f"att_ps{g}")
                      for g in range(2)]
            for g in range(2):
                nc.tensor.matmul(att_ps[g][:], ones_r,
                                 rows[0:1, 384 + g * 128:384 + (g + 1) * 128],
                                 start=True, stop=False)
            for g in range(2):
                gs = slice(g * 128, (g + 1) * 128)
                # Logical wait pushes the v-conv matmuls AFTER the scores
                # matmuls in the scheduler's PE stream: the scheduler
                # mis-estimates wv's DMA arrival as early and would otherwise
                # park the (earlier-ready) scores chain behind them.
                with tc.tile_wait_until(0.008 + 0.002 * g):
                    for c in range(NCHUNK):
                        nc.tensor.matmul(
                            v_ps[g][:], xbt[:, c * 128:(c + 1) * 128],
                            wvt[:, g * 2048 + c * 128:g * 2048 + (c + 1) * 128],
                            start=(c == 0), stop=(c == NCHUNK - 1))
                veng = nc.gpsimd if g == 0 else nc.vector
                veng.tensor_tensor(vpt[:, gs], v_ps[g][:], posa[:, gs],
                                   op=mybir.AluOpType.add)
                # att matmuls pinned after ALL v-conv chunks in the PE
                # stream: they become ready later than the last chunks and
                # would otherwise block them in the in-order pipeline.
                with tc.tile_wait_until(0.013 + 0.001 * g):
                    nc.tensor.matmul(att_ps[g][:], att[:], vpt[:, gs],
                                     start=False, stop=True)
                nc.vector.tensor_copy(xloc[:, gs], att_ps[g][:])
                eng = nc.scalar if g == 0 else nc.sync
                eng.dma_start(out_d.ap()[:, gs], xloc[:, gs])

    nc.compile()
    return nc


def _fold_bn(w, b, g, beta, m, v):
    s = g / np.sqrt(v + EPS)
    return (w * s[:, None]).astype(np.float32), (s * (b - m) + beta).astype(np.float32)


def _prep(inputs):
    """Host-side prep: BN folds, fp8/bf16 packing, per-core input maps."""
    import ml_dtypes
    bf = ml_dtypes.bfloat16
    f8 = ml_dtypes.float8_e4m3

    inp = {k: np.asarray(v, dtype=np.float32) for k, v in inputs.items()}
    x, pos = inp["x"], inp["pos"]
    wk, bk = _fold_bn(inp["wk"], inp["bk"], inp["gk"], inp["betak"], inp["mk"], inp["vk"])
    wq, bq = _fold_bn(inp["wq"], inp["bq"], inp["gq"], inp["betaq"], inp["mq"], inp["vq"])
    wv, bv = _fold_bn(inp["wv"], inp["bv"], inp["gv"], inp["betav"], inp["mv"], inp["vv"])
    wd, bd = _fold_bn(inp["wd"], inp["bd"], inp["gd"], inp["betad"], inp["md"], inp["vd"])
    so = (inp["go"] / np.sqrt(inp["vo"] + EPS)).astype(np.float32)
    beta_o = (inp["beto"] - inp["mo"] * so).astype(np.float32)
    wv = wv * so[:, None]
    bv = bv * so  # folded into posA below

    def pack_dr_a(w):
        # A-layout rhs for DoubleRow: [p, (cp, t, 256o)]; w is [256o, 2048in]
        wt = w.T.reshape(NPAIR, 2, 128, 256).transpose(2, 0, 1, 3).reshape(128, -1)
        return np.ascontiguousarray(wt).astype(f8)

    def pack_dr_b(w):
        # B-layout lhsT for DoubleRow: [p, (h, cp, t, 128o)]; w is [256o, 2048in]
        wt = w.T.reshape(NPAIR, 2, 128, 2, 128).transpose(2, 3, 0, 1, 4).reshape(128, -1)
        return np.ascontiguousarray(wt).astype(f8)

    def pack_hmaj(w):
        # outch-half-major bf16 (v conv): [p, (h, c, 128o)]
        wt = w.T.reshape(NCHUNK, 128, 2, 128).transpose(1, 2, 0, 3).reshape(128, -1)
        return np.ascontiguousarray(wt).astype(bf)

    w_packed = {"wd": pack_dr_a(wd), "wk": pack_dr_b(wk), "wq": pack_dr_b(wq),
                "wv": pack_hmaj(wv)}

    p_idx = np.arange(128)
    R = np.zeros((128, 8), np.float32)
    R[p_idx, (p_idx // 64) * 4 + (p_idx % 16) // 4] = 1.0
    pix_patch = (p_idx // 64) * 4 + (p_idx % 64) // 16
    blk_ind = (pix_patch[None, :] == np.arange(8)[:, None]).astype(np.float32)

    rows = np.zeros((1, ROWS_LEN), np.float32)
    rows[0, _R_ONES] = 1.0
    rows[0, _R_BD] = bd
    rows[0, _R_BETA] = beta_o
    rows[0, _R_BKQ] = np.concatenate([bk, bq])
    rows = rows.astype(bf)

    mask9 = np.zeros((9, 256), np.float32)
    mask9[0, 0:128] = 1.0
    mask9[0, 128:256] = -MASK_NEG
    mask9[1:9, 0:128] = blk_ind
    mask9[1:9, 128:256] = blk_ind * MASK_NEG
    mask9 = mask9.astype(bf)

    units = [(b, i) for b in range(B) for i in range(P)]
    in_maps = []
    for core in range(N_CORES):
        cu = units[2 * core:2 * core + 2]
        x_sb = np.empty((128, NCHUNK, 128), np.float32)
        pos_A = np.empty((128, 256), np.float32)
        posb_sb = np.empty((128, 256), np.float32)
        for u, (b, i) in enumerate(cu):
            # [c, ph, jp, pw] -> patch-major pixel (jp, ph, pw)
            xs = x[b, :, 4 * i:4 * i + 4, :].reshape(D_IN, 4, 4, 4)
            xs = xs.transpose(0, 2, 1, 3).reshape(D_IN, 64)
            x_sb[:, :, 64 * u:64 * u + 64] = xs.reshape(NCHUNK, 128, 64).transpose(1, 0, 2)
            ps_ = pos[b, :, 4 * i:4 * i + 4, :].reshape(D, 4, 4, 4).transpose(0, 2, 1, 3).reshape(D, 64)
            pos_A[64 * u:64 * u + 64, :] = ps_.T
            posb_sb[:, 64 * u:64 * u + 64] = ps_[0:128]
            posb_sb[:, 128 + 64 * u:128 + 64 * u + 64] = ps_[128:256]
        pos_A_sov = (pos_A * so[None, :] + bv[None, :]).astype(np.float32)
        xb = np.ascontiguousarray(x_sb.reshape(128, -1)).astype(bf)
        x8 = xb.astype(f8)  # fp8(bf16(x)): matches the CPU-validated chain
        combo = np.concatenate(
            [pos_A_sov, posb_sb, R], axis=1).astype(bf)
        in_maps.append({
            "x8": x8, "xb": xb,
            "wd": w_packed["wd"], "wk": w_packed["wk"],
            "wq": w_packed["wq"], "wv": w_packed["wv"],
            "combo": combo, "rows": rows, "mask9": mask9,
        })
    return in_maps, units


def _run_device(nc, in_maps):
    from concourse.bass_utils import run_bass_kernel_spmd
    return run_bass_kernel_spmd(nc, in_maps, list(range(N_CORES))).results


def _subproc_main(inp_path, out_path):
    import pickle
    with open(inp_path, "rb") as f:
        in_maps = pickle.load(f)
    nc = _build_program()
    res = _run_device(nc, in_maps)
    with open(out_path, "wb") as f:
        pickle.dump(res, f)


def _run_via_subprocess(in_maps):
    import pickle
    import subprocess
    import tempfile
    here = os.path.dirname(os.path.abspath(__file__))
    last = None
    for _ in range(2):
        with tempfile.TemporaryDirectory() as td:
            inp = os.path.join(td, "in.pkl")
            outp = os.path.join(td, "out.pkl")
            with open(inp, "wb") as f:
                pickle.dump(in_maps, f)
            code = (f"import sys; sys.path.insert(0, {here!r}); "
                    f"import kernel; kernel._subproc_main({inp!r}, {outp!r})")
            try:
                r = subprocess.run([sys.executable, "-c", code], timeout=1800)
                if r.returncode == 0 and os.path.exists(outp):
                    with open(outp, "rb") as f:
                        return pickle.load(f)
                last = RuntimeError(f"subprocess rc={r.returncode}")
            except Exception as e:  # noqa: BLE001
                last = e
    raise RuntimeError(f"device execution failed after retries: {last}")


def kernel(**inputs) -> np.ndarray:
    key = ("prog", "mixed_fp8")
    if key not in _CACHE:
        _CACHE[key] = _build_program()
    nc = _CACHE[key]

    in_maps, units = _prep(inputs)
    try:
        results = _run_device(nc, in_maps)
    except Exception:
        # A crashed NEFF execution can poison this process's jax runtime
        # (NRT_EXEC_UNIT_UNRECOVERABLE); a fresh process recovers reliably.
        results = _run_via_subprocess(in_maps)

    x_loc = np.zeros((B, D, HW, HW), np.float32)
    for core in range(N_CORES):
        xl = np.asarray(results[core]["xloc"], dtype=np.float32)  # [128 pix, 256 c]
        for u, (b, i) in enumerate(units[2 * core:2 * core + 2]):
            blk = xl[64 * u:64 * u + 64, :].reshape(4, 4, 4, D).transpose(3, 1, 0, 2)
            x_loc[b, :, 4 * i:4 * i + 4, :] = blk.reshape(D, 4, 16)
    return np.concatenate([np.asarray(inputs["x"], np.float32), x_loc], axis=1)
